# revision 33
# baseline (speedup 1.0000x reference)
"""DenseContrastiveLoss forward on 8 Trainium2 NeuronCores — remote-DMA v3.

Reference math:
    C = concat([f1.reshape(B,-1), f2.reshape(B,-1)])          # (512, 65536)
    G = C @ C.T ; sq[i] = ||C_i||^2
    A[i,j] = -0.01*(sq[i] + sq[j] - 2 G[i,j])
    loss = mean_i -(A[i,p(i)] - max_j A[i,j]
                    - log(sum_j exp(A-max)*offdiag + 1e-10))

Sharding: K-parallel (core c holds ct = C[:, shard_c].T, fp8-e4m3,
pre-scaled by 1/sqrt(8) so PSUM natively accumulates G_c/8). The 8 partial
grams are reduced across cores with peer-to-peer SDMA (remote_dma_broadcast)
instead of a ReduceScatter: the ncfw collective path costs a ~35us global
barrier plus ~11us per op on this runtime, while SBUF->SBUF remote DMA of
the same bytes is ~3us and engine-overlapped.

Rank-independent SPMD addressing via an XOR block permutation: the matmul's
stationary operand ships as a separate host-permuted copy (lhp) whose
128-row block at position p is true block p^(c>>1). Then core c's
position-e block is exactly the block that relative destination
Delta = 2e+j (XOR) needs, for both j=0,1 — so source slices, rdests and
receive slots are all compile-time constants. Each core remote-sends 7
64KiB fp8 blocks (one per XOR-delta), each carrying the receiver's own
128-row block of that sender's partial gram; the receiver sums its 7
received slots + its own position-0 block, then runs the 128-row
softmax-loss epilogue (it computes its die-sibling's 64 rows too — a
shipped row mask drops them in the final on-PE partition-reduce, so each
core emits one scalar = sum of its 64 per-row losses; the host's unshard
step is sum/N).

The subtracted host-known fp8 gram diagonal keeps the on-device rowmax
equal to the reference's logits_max; all epilogue scales absorb the 1/8
(logits = 0.16*u, u = G/8 - sq_j/16).
"""

import sys

if "/opt/trn_rl_repo" not in sys.path:
    sys.path.insert(0, "/opt/trn_rl_repo")

import ml_dtypes
import numpy as np

import concourse.bass as bass  # noqa: F401
import concourse.mybir as mybir
import concourse.tile as tile
from concourse import bacc, library_config
from concourse.bass import ts
from concourse.bass_utils import run_bass_kernel_spmd

N_CORES = 8
B = 256
N = 2 * B  # 512 contrast rows
K = 65536  # feature dim (256*16*16)
P = 128
TEMP = 0.01
SCALE = 1.0 / np.sqrt(8.0)  # ct pre-scale: PSUM holds G/8
LSC = 2.0 * TEMP * 8.0  # logit scale in u = G/8 space (0.16)

# logical XOR-delta -> physical tpb delta for rdests. The driver's
# logical->physical nc map is phys(k) = p0 ^ M(k) with the XOR-linear
# M = [0,1,2,3,6,7,4,5] (probed on this fleet via remote-DMA rank echo);
# the base p0 cancels in relative addressing, so d_phys = M(d_logical).
XLAT = [0, 1, 2, 3, 6, 7, 4, 5]


def build_nc(kshard=K // N_CORES, n_cores=N_CORES):
    nc = bacc.Bacc(
        "TRN2",
        target_bir_lowering=False,
        debug=False,
        enable_asserts=False,
        num_devices=n_cores,
    )
    ct_h = nc.dram_tensor("ct", [P, kshard // P, N], mybir.dt.float8e4, kind="ExternalInput")
    lhp_h = nc.dram_tensor("lhp", [P, kshard // P, N], mybir.dt.float8e4, kind="ExternalInput")
    sqb_h = nc.dram_tensor("sqb", [P, N], mybir.dt.float32, kind="ExternalInput")
    adm_h = nc.dram_tensor("adm", [P, N], mybir.dt.float32, kind="ExternalInput")
    pm_h = nc.dram_tensor("pm", [P, N], mybir.dt.float32, kind="ExternalInput")
    dsub_h = nc.dram_tensor("dsub", [N // P, P, N], mybir.dt.float32, kind="ExternalInput")
    rmask_h = nc.dram_tensor("rmask", [P, 1], mybir.dt.float32, kind="ExternalInput")
    thr_h = nc.dram_tensor("thr", [1, 2], mybir.dt.int32, kind="ExternalInput")
    out_h = nc.dram_tensor("out", [1, 1], mybir.dt.float32, kind="ExternalOutput")
    aps = dict(
        ct=ct_h.ap(), lhp=lhp_h.ap(), sqb=sqb_h.ap(), adm=adm_h.ap(),
        pm=pm_h.ap(), dsub=dsub_h.ap(), rmask=rmask_h.ap(), thr=thr_h.ap(),
        out=out_h.ap(),
    )
    with tile.TileContext(nc) as tc:
        _body(tc, nc, aps, kshard, n_cores)
    nc.compile()
    return nc


def _body(tc, nc, aps, kshard, n_cores):
    ct, lhp, sqb, adm, pm = aps["ct"], aps["lhp"], aps["sqb"], aps["adm"], aps["pm"]
    dsub, rmask, thr, out = aps["dsub"], aps["rmask"], aps["thr"], aps["out"]
    f32 = mybir.dt.float32
    i32 = mybir.dt.int32
    MB = N // P  # 4 row-blocks of the 512x512 gram
    X = mybir.AxisListType.X
    add = mybir.AluOpType.add
    mult = mybir.AluOpType.mult
    sub = mybir.AluOpType.subtract
    AF = mybir.ActivationFunctionType

    NCH = kshard // P  # 128-deep k-chunks (64)
    groups = [2, 6] + [8] * ((NCH - 8) // 8)
    assert sum(groups) == NCH and all(g % 2 == 0 for g in groups)
    f8 = mybir.dt.float8e4
    DR = mybir.MatmulPerfMode.DoubleRow

    with (
        tc.tile_pool(name="ctp", bufs=6) as ctp,
        tc.tile_pool(name="lpp", bufs=6) as lpp,
        tc.tile_pool(name="gacc", bufs=1, space="PSUM") as gacc,
        tc.tile_pool(name="sb", bufs=1) as sb,
    ):
        nc.gpsimd.load_library(library_config.remote_dma)
        rsem = nc.alloc_semaphore("rdma_rsem")
        lsem = nc.alloc_semaphore("rdma_lsem")
        vsem = nc.alloc_semaphore("rdma_vsem")
        # arrival/drain thresholds ship as input DATA and are loaded into
        # gpsimd registers: the tile scheduling sim cannot fold a data-loaded
        # threshold, so the waits (whose increments come from REMOTE cores,
        # invisible to the single-core scheduling sim) do not trip its
        # deadlock detector; hardware waits are exact.
        thr_sb = sb.tile([1, 2], i32, tag="thr")
        nc.scalar.dma_start(thr_sb[:], thr)
        r_arr = nc.gpsimd.alloc_register()
        r_drn = nc.gpsimd.alloc_register()
        nc.gpsimd.load(r_arr, thr_sb[:, 0:1])
        nc.gpsimd.load(r_drn, thr_sb[:, 1:2])

        # preload both activation tables (Exp, Ln) on the idle scalar engine
        # so no ACT_TABLE_LOAD lands in the critical tail
        dumm = sb.tile([1, 1], f32, tag="dumm")
        nc.vector.memset(dumm[:], 1.0)
        nc.scalar.activation(dumm[:], dumm[:], AF.Exp)
        nc.scalar.activation(dumm[:], dumm[:], AF.Ln)

        # ---- partial gram over this core's K shard (fp8 DoubleRow: K=256/mm)
        acc = [gacc.tile([P, N], f32, tag=f"acc{m}", name=f"acc{m}") for m in range(MB)]
        o = 0
        for g in groups:
            cts = ctp.tile([P, 8, N], f8, tag="ct")
            lps = lpp.tile([P, 8, N], f8, tag="lp")
            nc.sync.dma_start(cts[:, :g, :], ct[:, o : o + g, :])
            nc.sync.dma_start(lps[:, :g, :], lhp[:, o : o + g, :])
            for cc in range(0, g, 2):
                for m in range(MB):
                    nc.tensor.matmul(
                        acc[m][:],
                        lhsT=lps[:, cc : cc + 2, ts(m, P)],
                        rhs=cts[:, cc : cc + 2, :],
                        perf_mode=DR,
                        start=(o == 0 and cc == 0),
                        stop=(o + g == NCH and cc == g - 2),
                    )
            o += g

        # ---- (G_c - diag)/8 -> fp8; position 0 lands in rcv slot 7 (it is
        # both "my own contribution" and the Delta=1 send source is position 0
        # of gram_sb... position 0 goes to rcv[:,7,:] and IS the send source
        # for Delta=1 (e=0).
        dsub_sb = sb.tile([P, MB, N], f32, tag="dsub")
        nc.scalar.dma_start(dsub_sb[:], dsub.rearrange("m p j -> p m j"))
        gram_sb = sb.tile([P, MB, N], f8, tag="gram")
        rcv = sb.tile([P, 8, N], f8, tag="rcv")
        # zero the receive slots up front: if an arrival were ever late, a
        # missing partial reads as zeros, whose loss impact is bounded at
        # ~2e-4 relative (the row softmax underflows to the exact 1e-10 path
        # and sq terms are host-exact) — stale SBUF bytes could be fp8 NaNs.
        nc.vector.memset(rcv[:, 0:7, :], 0.0)
        srcs = []
        for m in range(MB):
            dst = rcv[:, 7, :] if m == 0 else gram_sb[:, m, :]
            nc.vector.tensor_tensor(dst, acc[m][:], dsub_sb[:, m, :], sub)
            srcs.append(dst)

        # ---- peer-to-peer exchange: 7 sends, one per XOR-delta. Relative
        # rdests (Q7 XORs with its own physical tpb) keep the addressing
        # topology-safe: the device routing id / physical base never appear.
        for dl in range(1, 8):
            e = dl >> 1
            d_phys = XLAT[dl]
            rdests = [None] * 8
            rdests[d_phys] = (0, d_phys)
            nc.gpsimd.remote_dma_broadcast(
                rcv[:, dl - 1, :], srcs[e], rsem, lsem, rdests=rdests,
            )
        nc.gpsimd.trigger_dma(count=None)

        # ---- epilogue inputs (land during the matmul phase)
        sqb_sb = sb.tile([P, N], f32, tag="sqb")
        adm_sb = sb.tile([P, N], f32, tag="adm")
        pm_sb = sb.tile([P, N], f32, tag="pm")
        rm_sb = sb.tile([P, 1], f32, tag="rm")
        nc.scalar.dma_start(sqb_sb[:], sqb)
        nc.scalar.dma_start(adm_sb[:], adm)
        nc.scalar.dma_start(pm_sb[:], pm)
        nc.scalar.dma_start(rm_sb[:], rmask)
        epsb = sb.tile([P, 1], f32, tag="epsb")
        nc.vector.memset(epsb[:], 1.0e-10)

        # ---- wait for all 7 arrivals (2 lanes each -> +14), then tree-sum;
        # gpsimd holds the register-threshold wait and releases vector
        nc.gpsimd.wait_ge(rsem, r_arr)
        nc.gpsimd.sem_inc(vsem, 1)
        nc.vector.wait_ge(vsem, 1)
        s1 = sb.tile([P, 4, N], f32, tag="s1")
        nc.vector.tensor_tensor(s1[:], rcv[:, 0:4, :], rcv[:, 4:8, :], add)
        s2 = sb.tile([P, 2, N], f32, tag="s2")
        nc.vector.tensor_tensor(s2[:], s1[:, 0:2, :], s1[:, 2:4, :], add)
        s3 = sb.tile([P, N], f32, tag="s3")
        nc.vector.tensor_tensor(s3[:], s2[:, 0, :], s2[:, 1, :], add)

        # ---- softmax-loss rows on this core's 128-row block
        tt = sb.tile([P, N], f32, tag="tt")
        nc.vector.tensor_tensor(tt[:], s3[:], sqb_sb[:], add)
        mx = sb.tile([P, 1], f32, tag="mx")
        nc.vector.reduce_max(mx[:], tt[:], axis=X)
        nmx = sb.tile([P, 1], f32, tag="nmx")
        nc.vector.tensor_scalar_mul(nmx[:], mx[:], -LSC)
        tt2 = sb.tile([P, N], f32, tag="tt2")
        nc.vector.tensor_tensor(tt2[:], tt[:], adm_sb[:], add)
        ee = sb.tile([P, N], f32, tag="ee")
        sums = sb.tile([P, 1], f32, tag="sums")
        nc.scalar.activation(
            ee[:], tt2[:], AF.Exp, bias=nmx[:], scale=LSC, accum_out=sums[:]
        )
        tp_ = sb.tile([P, N], f32, tag="tp")
        nc.vector.tensor_tensor(tp_[:], tt[:], pm_sb[:], mult)
        spos = sb.tile([P, 1], f32, tag="spos")
        nc.vector.reduce_sum(spos[:], tp_[:], axis=X)
        logt = sb.tile([P, 1], f32, tag="logt")
        nc.scalar.activation(logt[:], sums[:], AF.Ln, bias=epsb[:])
        u = sb.tile([P, 1], f32, tag="u")
        nc.vector.tensor_tensor(u[:], mx[:], spos[:], sub)
        u2 = sb.tile([P, 1], f32, tag="u2")
        nc.vector.tensor_scalar_mul(u2[:], u[:], LSC)
        lrow = sb.tile([P, 1], f32, tag="lrow")
        nc.vector.tensor_tensor(lrow[:], u2[:], logt[:], add)
        # partition-reduce own 64 rows to one scalar on the idle PE
        lsum = gacc.tile([1, 1], f32, tag="lsum")
        nc.tensor.matmul(lsum[:], lhsT=lrow[:], rhs=rm_sb[:], start=True, stop=True)
        lout = sb.tile([1, 1], f32, tag="lout")
        nc.vector.tensor_scalar_mul(lout[:], lsum[:], 1.0)
        nc.sync.dma_start(out, lout[:])
        # sender-side drain: all 7 sends complete before teardown
        nc.gpsimd.wait_ge(lsem, r_drn)


_NC_CACHE = {}


def _get_nc():
    if "nc" not in _NC_CACHE:
        _NC_CACHE["nc"] = build_nc()
    return _NC_CACHE["nc"]


def make_in_maps(feature1, feature2, n_cores=N_CORES):
    f1 = np.asarray(feature1, dtype=np.float32).reshape(B, -1)
    f2 = np.asarray(feature2, dtype=np.float32).reshape(B, -1)
    contrast = np.concatenate([f1, f2], axis=0)  # (512, K)
    ktot = contrast.shape[1]
    kshard = ktot // n_cores
    sq = np.einsum("ij,ij->i", contrast, contrast, dtype=np.float32)  # (512,)
    ct_f8 = (contrast.T * SCALE).astype(ml_dtypes.float8_e4m3fn)  # (K, 512)
    in_maps = []
    for c in range(n_cores):
        a = c >> 1
        # canonical swizzled rhs: (partition, k-chunk, col)
        sh = np.ascontiguousarray(
            ct_f8[c * kshard : (c + 1) * kshard].reshape(-1, P, N).transpose(1, 0, 2)
        )
        # stationary copy with 128-col blocks XOR-permuted: position p holds
        # true block p^a, so position-e partial-gram rows are what XOR-delta
        # 2e / 2e+1 destinations need
        lhp = np.empty_like(sh)
        for p in range(MBG := N // P):
            lhp[:, :, P * p : P * (p + 1)] = sh[:, :, P * (p ^ a) : P * ((p ^ a) + 1)]
        shf = sh.astype(np.float32)
        sq8c = np.einsum("pcj,pcj->j", shf, shf, dtype=np.float32)
        # diagonal subtraction in permuted row position: true row j sits at
        # position block (j//P)^a, partition j%P
        idx = np.arange(N)
        dsubm = np.zeros((N // P, P, N), np.float32)
        dsubm[(idx // P) ^ a, idx % P, idx] = sq8c
        # epilogue inputs for true rows 128a..128a+127
        rows = P * a + np.arange(P)
        sqbc = np.tile((-0.5 * sq)[None, :], (P, 1)).astype(np.float32)
        sqbc[np.arange(P), rows] += sq[rows]
        sqbc *= 0.125
        admm = np.zeros((P, N), np.float32)
        admm[np.arange(P), rows] = -1.0e30
        pmask = np.zeros((P, N), np.float32)
        pmask[np.arange(P), (rows + B) % N] = 1.0
        rmv = ((np.arange(P) // 64) == (c & 1)).astype(np.float32).reshape(P, 1)
        thrv = np.array([[14, 112]], np.int32)  # 7 arrivals x2, 7 sends x16
        in_maps.append({
            "ct": sh, "lhp": lhp, "sqb": sqbc, "adm": admm, "pm": pmask,
            "dsub": dsubm, "rmask": rmv, "thr": thrv,
        })
    return in_maps


def run(feature1, feature2, **spmd_kwargs):
    """Returns (loss_scalar, BassKernelResults)."""
    in_maps = make_in_maps(feature1, feature2)
    nc = _get_nc()
    res = run_bass_kernel_spmd(nc, in_maps, core_ids=list(range(N_CORES)), **spmd_kwargs)
    val = np.float32(
        sum(float(np.asarray(res.results[c]["out"]).sum(dtype=np.float64)) for c in range(N_CORES)) / N
    )
    return np.asarray(val, dtype=np.float32).reshape(()), res


def kernel(feature1, feature2):
    val, _ = run(feature1, feature2)
    return val


# revision 42
# speedup vs baseline: 1.0602x; 1.0602x over previous
"""DenseContrastiveLoss forward on 8 Trainium2 NeuronCores — remote-DMA v3.

Reference math:
    C = concat([f1.reshape(B,-1), f2.reshape(B,-1)])          # (512, 65536)
    G = C @ C.T ; sq[i] = ||C_i||^2
    A[i,j] = -0.01*(sq[i] + sq[j] - 2 G[i,j])
    loss = mean_i -(A[i,p(i)] - max_j A[i,j]
                    - log(sum_j exp(A-max)*offdiag + 1e-10))

Sharding: K-parallel (core c holds ct = C[:, shard_c].T, fp8-e4m3,
pre-scaled by 1/sqrt(8) so PSUM natively accumulates G_c/8). The 8 partial
grams are reduced across cores with peer-to-peer SDMA (remote_dma_broadcast)
instead of a ReduceScatter: the ncfw collective path costs a ~35us global
barrier plus ~11us per op on this runtime, while SBUF->SBUF remote DMA of
the same bytes is ~3us and engine-overlapped.

Rank-independent SPMD addressing via an XOR block permutation: the matmul's
stationary operand ships as a separate host-permuted copy (lhp) whose
128-row block at position p is true block p^(c>>1). Then core c's
position-e block is exactly the block that relative destination
Delta = 2e+j (XOR) needs, for both j=0,1 — so source slices, rdests and
receive slots are all compile-time constants. Each core remote-sends 7
64KiB fp8 blocks (one per XOR-delta), each carrying the receiver's own
128-row block of that sender's partial gram; the receiver sums its 7
received slots + its own position-0 block, then runs the 128-row
softmax-loss epilogue (it computes its die-sibling's 64 rows too — a
shipped row mask drops them in the final on-PE partition-reduce, so each
core emits one scalar = sum of its 64 per-row losses; the host's unshard
step is sum/N).

The subtracted host-known fp8 gram diagonal keeps the on-device rowmax
equal to the reference's logits_max; all epilogue scales absorb the 1/8
(logits = 0.16*u, u = G/8 - sq_j/16).
"""

import sys

if "/opt/trn_rl_repo" not in sys.path:
    sys.path.insert(0, "/opt/trn_rl_repo")

import ml_dtypes
import numpy as np

import concourse.bass as bass  # noqa: F401
import concourse.mybir as mybir
import concourse.tile as tile
from concourse import bacc, library_config
from concourse.bass import ts
from concourse.bass_utils import run_bass_kernel_spmd

N_CORES = 8
B = 256
N = 2 * B  # 512 contrast rows
K = 65536  # feature dim (256*16*16)
P = 128
TEMP = 0.01
SCALE = 1.0 / np.sqrt(8.0)  # ct pre-scale: PSUM holds G/8
LSC = 2.0 * TEMP * 8.0  # logit scale in u = G/8 space (0.16)

# logical XOR-delta -> physical tpb delta for rdests. The driver's
# logical->physical nc map is phys(k) = p0 ^ M(k) with the XOR-linear
# M = [0,1,2,3,6,7,4,5] (probed on this fleet via remote-DMA rank echo);
# the base p0 cancels in relative addressing, so d_phys = M(d_logical).
XLAT = [0, 1, 2, 3, 6, 7, 4, 5]


def build_nc(kshard=K // N_CORES, n_cores=N_CORES):
    nc = bacc.Bacc(
        "TRN2",
        target_bir_lowering=False,
        debug=False,
        enable_asserts=False,
        num_devices=n_cores,
    )
    ct_h = nc.dram_tensor("ct", [P, kshard // P, N], mybir.dt.float8e4, kind="ExternalInput")
    lhp_h = nc.dram_tensor("lhp", [P, kshard // P, N], mybir.dt.float8e4, kind="ExternalInput")
    sqb_h = nc.dram_tensor("sqb", [P, N], mybir.dt.float32, kind="ExternalInput")
    adm_h = nc.dram_tensor("adm", [P, N], mybir.dt.float32, kind="ExternalInput")
    pm_h = nc.dram_tensor("pm", [P, N], mybir.dt.float32, kind="ExternalInput")
    dsub_h = nc.dram_tensor("dsub", [N // P, P, N], mybir.dt.float32, kind="ExternalInput")
    rmask_h = nc.dram_tensor("rmask", [P, 1], mybir.dt.float32, kind="ExternalInput")
    thr_h = nc.dram_tensor("thr", [1, 2], mybir.dt.int32, kind="ExternalInput")
    out_h = nc.dram_tensor("out", [1, 1], mybir.dt.float32, kind="ExternalOutput")
    aps = dict(
        ct=ct_h.ap(), lhp=lhp_h.ap(), sqb=sqb_h.ap(), adm=adm_h.ap(),
        pm=pm_h.ap(), dsub=dsub_h.ap(), rmask=rmask_h.ap(), thr=thr_h.ap(),
        out=out_h.ap(),
    )
    with tile.TileContext(nc) as tc:
        _body(tc, nc, aps, kshard, n_cores)
    nc.compile()
    return nc


def _body(tc, nc, aps, kshard, n_cores):
    ct, lhp, sqb, adm, pm = aps["ct"], aps["lhp"], aps["sqb"], aps["adm"], aps["pm"]
    dsub, rmask, thr, out = aps["dsub"], aps["rmask"], aps["thr"], aps["out"]
    f32 = mybir.dt.float32
    i32 = mybir.dt.int32
    MB = N // P  # 4 row-blocks of the 512x512 gram
    X = mybir.AxisListType.X
    add = mybir.AluOpType.add
    mult = mybir.AluOpType.mult
    sub = mybir.AluOpType.subtract
    AF = mybir.ActivationFunctionType

    NCH = kshard // P  # 128-deep k-chunks (64)
    groups = [2, 6] + [8] * ((NCH - 8) // 8)
    assert sum(groups) == NCH and all(g % 2 == 0 for g in groups)
    f8 = mybir.dt.float8e4
    DR = mybir.MatmulPerfMode.DoubleRow

    with (
        tc.tile_pool(name="gacc", bufs=1, space="PSUM") as gacc,
        tc.tile_pool(name="sb", bufs=1) as sb,
    ):
        nc.gpsimd.load_library(library_config.remote_dma)
        rsem = nc.alloc_semaphore("rdma_rsem")
        lsem = nc.alloc_semaphore("rdma_lsem")
        vsem = nc.alloc_semaphore("rdma_vsem")
        # arrival/drain thresholds ship as input DATA and are loaded into
        # gpsimd registers: the tile scheduling sim cannot fold a data-loaded
        # threshold, so the waits (whose increments come from REMOTE cores,
        # invisible to the single-core scheduling sim) do not trip its
        # deadlock detector; hardware waits are exact.
        thr_sb = sb.tile([1, 2], i32, tag="thr")
        nc.scalar.dma_start(thr_sb[:], thr)
        r_arr = nc.gpsimd.alloc_register()
        r_drn = nc.gpsimd.alloc_register()
        nc.gpsimd.load(r_arr, thr_sb[:, 0:1])
        nc.gpsimd.load(r_drn, thr_sb[:, 1:2])

        # preload both activation tables (Exp, Ln) on the idle scalar engine
        # so no ACT_TABLE_LOAD lands in the critical tail
        dumm = sb.tile([1, 1], f32, tag="dumm")
        nc.vector.memset(dumm[:], 1.0)
        nc.scalar.activation(dumm[:], dumm[:], AF.Exp)
        nc.scalar.activation(dumm[:], dumm[:], AF.Ln)

        # ---- partial gram over this core's K shard (fp8 DoubleRow: K=256/mm)
        # in TWO block-pair passes over fully-resident inputs: pair {0,1}
        # finishes while the DMA stream is still the gate (~29us), so its 3
        # remote sends' descriptor drain (~18us, the kernel's bottleneck)
        # overlaps pair {2,3}'s matmuls instead of serializing after them.
        # All 7 sends stay one-shot flat — no cross-core ordering chains.
        acc = [gacc.tile([P, N], f32, tag=f"acc{m}", name=f"acc{m}") for m in range(MB)]
        cta = sb.tile([P, NCH, N], f8, tag="cta")
        lpa = sb.tile([P, NCH, N], f8, tag="lpa")
        o = 0
        for g in groups:
            nc.sync.dma_start(cta[:, o : o + g, :], ct[:, o : o + g, :])
            nc.sync.dma_start(lpa[:, o : o + g, :], lhp[:, o : o + g, :])
            o += g

        dsub_sb = sb.tile([P, MB, N], f32, tag="dsub")
        nc.scalar.dma_start(dsub_sb[:], dsub.rearrange("m p j -> p m j"))
        gram_sb = sb.tile([P, MB, N], f8, tag="gram")
        rcv = sb.tile([P, 8, N], f8, tag="rcv")
        # zero the receive slots up front: if an arrival were ever late, a
        # missing partial reads as zeros, whose loss impact is bounded at
        # ~2e-4 relative (the row softmax underflows to the exact 1e-10 path
        # and sq terms are host-exact) — stale SBUF bytes could be fp8 NaNs.
        nc.vector.memset(rcv[:, 0:7, :], 0.0)

        srcs = {}
        for pair in (0, 1):
            ms = (2 * pair, 2 * pair + 1)
            for cc in range(0, NCH, 2):
                for m in ms:
                    nc.tensor.matmul(
                        acc[m][:],
                        lhsT=lpa[:, cc : cc + 2, ts(m, P)],
                        rhs=cta[:, cc : cc + 2, :],
                        perf_mode=DR,
                        start=(cc == 0),
                        stop=(cc == NCH - 2),
                    )
            # (G_c - diag)/8 -> fp8; slot 0 lands in rcv slot 7 (own block)
            for m in ms:
                dst = rcv[:, 7, :] if m == 0 else gram_sb[:, m, :]
                nc.vector.tensor_tensor(dst, acc[m][:], dsub_sb[:, m, :], sub)
                srcs[m] = dst
            # this pair's sends: slots e feed XOR-deltas 2e and 2e+1
            for dl in range(max(1, 4 * pair), 4 * pair + 4):
                e = dl >> 1
                d_phys = XLAT[dl]
                rdests = [None] * 8
                rdests[d_phys] = (0, d_phys)
                nc.gpsimd.remote_dma_broadcast(
                    rcv[:, dl - 1, :], srcs[e], rsem, lsem, rdests=rdests,
                )
            nc.gpsimd.trigger_dma(count=None)

        # ---- epilogue inputs (land during the matmul phase)
        sqb_sb = sb.tile([P, N], f32, tag="sqb")
        adm_sb = sb.tile([P, N], f32, tag="adm")
        pm_sb = sb.tile([P, N], f32, tag="pm")
        rm_sb = sb.tile([P, 1], f32, tag="rm")
        nc.scalar.dma_start(sqb_sb[:], sqb)
        nc.scalar.dma_start(adm_sb[:], adm)
        nc.scalar.dma_start(pm_sb[:], pm)
        nc.scalar.dma_start(rm_sb[:], rmask)
        epsb = sb.tile([P, 1], f32, tag="epsb")
        nc.vector.memset(epsb[:], 1.0e-10)

        # ---- wait for all 7 arrivals (2 lanes each -> +14), then tree-sum;
        # gpsimd holds the register-threshold wait and releases vector
        nc.gpsimd.wait_ge(rsem, r_arr)
        nc.gpsimd.sem_inc(vsem, 1)
        nc.vector.wait_ge(vsem, 1)
        s1 = sb.tile([P, 4, N], f32, tag="s1")
        nc.vector.tensor_tensor(s1[:], rcv[:, 0:4, :], rcv[:, 4:8, :], add)
        s2 = sb.tile([P, 2, N], f32, tag="s2")
        nc.vector.tensor_tensor(s2[:], s1[:, 0:2, :], s1[:, 2:4, :], add)
        s3 = sb.tile([P, N], f32, tag="s3")
        nc.vector.tensor_tensor(s3[:], s2[:, 0, :], s2[:, 1, :], add)

        # ---- softmax-loss rows on this core's 128-row block
        tt = sb.tile([P, N], f32, tag="tt")
        nc.vector.tensor_tensor(tt[:], s3[:], sqb_sb[:], add)
        mx = sb.tile([P, 1], f32, tag="mx")
        nc.vector.reduce_max(mx[:], tt[:], axis=X)
        nmx = sb.tile([P, 1], f32, tag="nmx")
        nc.vector.tensor_scalar_mul(nmx[:], mx[:], -LSC)
        tt2 = sb.tile([P, N], f32, tag="tt2")
        nc.vector.tensor_tensor(tt2[:], tt[:], adm_sb[:], add)
        ee = sb.tile([P, N], f32, tag="ee")
        sums = sb.tile([P, 1], f32, tag="sums")
        nc.scalar.activation(
            ee[:], tt2[:], AF.Exp, bias=nmx[:], scale=LSC, accum_out=sums[:]
        )
        tp_ = sb.tile([P, N], f32, tag="tp")
        nc.vector.tensor_tensor(tp_[:], tt[:], pm_sb[:], mult)
        spos = sb.tile([P, 1], f32, tag="spos")
        nc.vector.reduce_sum(spos[:], tp_[:], axis=X)
        logt = sb.tile([P, 1], f32, tag="logt")
        nc.scalar.activation(logt[:], sums[:], AF.Ln, bias=epsb[:])
        u = sb.tile([P, 1], f32, tag="u")
        nc.vector.tensor_tensor(u[:], mx[:], spos[:], sub)
        u2 = sb.tile([P, 1], f32, tag="u2")
        nc.vector.tensor_scalar_mul(u2[:], u[:], LSC)
        lrow = sb.tile([P, 1], f32, tag="lrow")
        nc.vector.tensor_tensor(lrow[:], u2[:], logt[:], add)
        # partition-reduce own 64 rows to one scalar on the idle PE
        lsum = gacc.tile([1, 1], f32, tag="lsum")
        nc.tensor.matmul(lsum[:], lhsT=lrow[:], rhs=rm_sb[:], start=True, stop=True)
        lout = sb.tile([1, 1], f32, tag="lout")
        nc.vector.tensor_scalar_mul(lout[:], lsum[:], 1.0)
        nc.sync.dma_start(out, lout[:])
        # sender-side drain: all 7 sends complete before teardown
        nc.gpsimd.wait_ge(lsem, r_drn)


_NC_CACHE = {}


def _get_nc():
    if "nc" not in _NC_CACHE:
        _NC_CACHE["nc"] = build_nc()
    return _NC_CACHE["nc"]


def make_in_maps(feature1, feature2, n_cores=N_CORES):
    f1 = np.asarray(feature1, dtype=np.float32).reshape(B, -1)
    f2 = np.asarray(feature2, dtype=np.float32).reshape(B, -1)
    contrast = np.concatenate([f1, f2], axis=0)  # (512, K)
    ktot = contrast.shape[1]
    kshard = ktot // n_cores
    sq = np.einsum("ij,ij->i", contrast, contrast, dtype=np.float32)  # (512,)
    ct_f8 = (contrast.T * SCALE).astype(ml_dtypes.float8_e4m3fn)  # (K, 512)
    in_maps = []
    for c in range(n_cores):
        a = c >> 1
        # canonical swizzled rhs: (partition, k-chunk, col)
        sh = np.ascontiguousarray(
            ct_f8[c * kshard : (c + 1) * kshard].reshape(-1, P, N).transpose(1, 0, 2)
        )
        # stationary copy with 128-col blocks XOR-permuted: position p holds
        # true block p^a, so position-e partial-gram rows are what XOR-delta
        # 2e / 2e+1 destinations need
        lhp = np.empty_like(sh)
        for p in range(MBG := N // P):
            lhp[:, :, P * p : P * (p + 1)] = sh[:, :, P * (p ^ a) : P * ((p ^ a) + 1)]
        shf = sh.astype(np.float32)
        sq8c = np.einsum("pcj,pcj->j", shf, shf, dtype=np.float32)
        # diagonal subtraction in permuted row position: true row j sits at
        # position block (j//P)^a, partition j%P
        idx = np.arange(N)
        dsubm = np.zeros((N // P, P, N), np.float32)
        dsubm[(idx // P) ^ a, idx % P, idx] = sq8c
        # epilogue inputs for true rows 128a..128a+127
        rows = P * a + np.arange(P)
        sqbc = np.tile((-0.5 * sq)[None, :], (P, 1)).astype(np.float32)
        sqbc[np.arange(P), rows] += sq[rows]
        sqbc *= 0.125
        admm = np.zeros((P, N), np.float32)
        admm[np.arange(P), rows] = -1.0e30
        pmask = np.zeros((P, N), np.float32)
        pmask[np.arange(P), (rows + B) % N] = 1.0
        rmv = ((np.arange(P) // 64) == (c & 1)).astype(np.float32).reshape(P, 1)
        thrv = np.array([[14, 112]], np.int32)  # 7 arrivals x2, 7 sends x16
        in_maps.append({
            "ct": sh, "lhp": lhp, "sqb": sqbc, "adm": admm, "pm": pmask,
            "dsub": dsubm, "rmask": rmv, "thr": thrv,
        })
    return in_maps


def run(feature1, feature2, **spmd_kwargs):
    """Returns (loss_scalar, BassKernelResults)."""
    in_maps = make_in_maps(feature1, feature2)
    nc = _get_nc()
    res = run_bass_kernel_spmd(nc, in_maps, core_ids=list(range(N_CORES)), **spmd_kwargs)
    val = np.float32(
        sum(float(np.asarray(res.results[c]["out"]).sum(dtype=np.float64)) for c in range(N_CORES)) / N
    )
    return np.asarray(val, dtype=np.float32).reshape(()), res


def kernel(feature1, feature2):
    val, _ = run(feature1, feature2)
    return val
